# revision 4
# baseline (speedup 1.0000x reference)
"""Pairwise KL divergence kernel for Trainium2, SPMD across 8 NeuronCores.

out[n, m] = sum_d a[n,d]*(log a[n,d] - log b[m,d])
          = ent[n] - (a @ log(b)^T)[n, m],  ent = rowsum(a * log a)

Sharding: a (and output rows) split 8 ways; b replicated.
Per core: a_shard (1024, 64), b (8192, 64) -> out_shard (1024, 8192).

Design (v2): the kernel is evac-bound, so everything else is stripped:
  - Host passes layout-permuted fp16 operands so ALL loads are contiguous
    big-descriptor DMAs and NO on-device transposes are needed:
      aT: a_shard^T, negated, duplicated on both partition halves
      bT: b^T with m-tiles 0..31 on partitions 0..63 (d axis) and m-tiles
          32..63 on partitions 64..127 -> after Ln this is the GEMM rhs
      an: a_shard in natural p-major layout (for the entropy term)
  - Device: lbT = Ln(bT) [ACT]; ent chain [ACT+DVE]; then 8 n-tiles x
    8 groups of dual half-array matmuls (K=64 stacked on both partition
    halves, concurrent on disjoint row groups); psum = -cross.
  - Output is stored as int8 with an affine code: q = rne(psum*inv + B),
    B[p,t] = ent*inv + cst (computed on device from qp input). Host
    dequantizes. The quant range [lo, hi] is a rigorous bound computed on
    the host (Cauchy-Schwarz + per-column extrema), so saturation never
    triggers; int8 + RNE gives ~0.4% rel err vs the 2e-2 gate.
  - Evac (the bottleneck: PSUM fp32 reads run at 1 elem/cycle/partition)
    is fused mult+add+quantize, FD=2048 per instruction, split DVE/ACT.
  - int8 output halves HBM store traffic vs fp16 (8 MiB/core).
"""

import numpy as np

N, M, D = 8192, 8192, 64
NCORES = 8
NSHARD = N // NCORES          # 1024 rows of a per core
NT = NSHARD // 128            # 8 n-tiles per core
K2 = M // 128 // 2            # 32 m-tile pairs (h0: tiles 0..31, h1: 32..63)
G = 8                         # groups per n-tile; each = 1024 out cols
NCH = 4                       # bT load/Ln chunks (8 pairs each)

OUT_MODE = "i8"               # "i8" | "f16"

_CACHE = {}


def _build(out_mode):
    from contextlib import ExitStack

    import concourse.bacc as bacc_mod
    import concourse.bass as bass
    import concourse.mybir as mybir
    import concourse.tile as tile

    FP32 = mybir.dt.float32
    F16 = mybir.dt.float16
    I8 = mybir.dt.int8
    AF = mybir.ActivationFunctionType
    ALU = mybir.AluOpType
    AX = mybir.AxisListType
    DT_OUT = I8 if out_mode == "i8" else F16

    nc = bacc_mod.Bacc()
    aT_d = nc.dram_tensor("aT", [128, NT, 128], F16, kind="ExternalInput")
    an_d = nc.dram_tensor("an", [128, NT, D], FP32, kind="ExternalInput")
    bT_d = nc.dram_tensor("bT", [128, K2, 128], F16, kind="ExternalInput")
    qp_d = nc.dram_tensor("qp", [128, 2], FP32, kind="ExternalInput")
    out_d = nc.dram_tensor("out", [NSHARD, M], DT_OUT, kind="ExternalOutput")

    with tile.TileContext(nc) as tc, ExitStack() as ctx:
        apool = ctx.enter_context(tc.tile_pool(name="apool", bufs=1))
        bpool = ctx.enter_context(tc.tile_pool(name="bpool", bufs=NCH))
        lbtp = ctx.enter_context(tc.tile_pool(name="lbtp", bufs=1))
        mmps = ctx.enter_context(tc.tile_pool(name="mmps", bufs=1, space="PSUM"))
        stage = ctx.enter_context(tc.tile_pool(name="stage", bufs=2))

        CH = K2 // NCH  # 8 pairs per chunk
        # -------- input DMAs, all issued up front on the sync sequencer ----
        b_raws = []
        for c in range(NCH):
            b_raw = bpool.tile([128, CH, 128], F16, tag="b_raw")
            b_raws.append(b_raw)
        nc.sync.dma_start(out=b_raws[0], in_=bT_d[:, 0:CH, :])
        aT = apool.tile([128, NT, 128], F16)
        nc.sync.dma_start(out=aT, in_=aT_d[:, :, :])
        an = apool.tile([128, NT, D], FP32)
        nc.sync.dma_start(out=an, in_=an_d[:, :, :])
        qp = apool.tile([128, 2], FP32)
        nc.sync.dma_start(out=qp, in_=qp_d[:, :])
        for c in range(1, NCH):
            nc.sync.dma_start(out=b_raws[c], in_=bT_d[:, c * CH : (c + 1) * CH, :])

        lbT = lbtp.tile([128, K2, 128], F16)

        # ACT program: Ln chunk0, then ent's Ln, then chunks 1-3, then evacs
        nc.scalar.activation(lbT[:, 0:CH, :], b_raws[0], AF.Ln)
        la = apool.tile([128, NT, D], FP32)
        nc.scalar.activation(la, an, AF.Ln)
        for c in range(1, NCH):
            nc.scalar.activation(
                lbT[:, c * CH : (c + 1) * CH, :], b_raws[c], AF.Ln
            )

        # DVE program: entropy chain + B, then evacs
        prod = apool.tile([128, NT, D], FP32)
        nc.vector.tensor_mul(prod, an, la)
        ent = apool.tile([128, NT], FP32)
        for t in range(NT):
            nc.vector.reduce_sum(ent[:, t : t + 1], prod[:, t, :], axis=AX.X)
        bias = apool.tile([128, NT], FP32)
        if out_mode == "i8":
            # B = ent*inv + cst
            nc.vector.tensor_scalar(
                bias, ent, qp[:, 0:1], qp[:, 1:2], ALU.mult, ALU.add
            )
        else:
            nc.vector.tensor_copy(bias, ent)

        # ---------------- main GEMM + fused evac ----------------
        # psum: all 8 banks as one tile; bank 2s+h = slot s, half h
        ps = mmps.tile([128, 8, 512], FP32)
        # out HBM: (t p) (h j i) with h=2 halves, j=4 dgroups, i=1024
        out_r = out_d[:, :].rearrange(
            "(t p) (h j i) -> t p h j i", p=128, h=2, i=1024
        )
        for t in range(NT):
            out_sb = stage.tile([128, 2, 4, 1024], DT_OUT, tag="out_sb")
            lhsT = aT[:, t, :]
            b_t = bias[:, t : t + 1]
            for j in range(4):  # double-groups: groups 2j, 2j+1
                for u in range(2):
                    g = 2 * j + u
                    s = g % 4
                    # h0: partitions 0-63 (m tiles 4g..4g+3), h1: 64-127
                    nc.tensor.matmul(
                        ps[:, 2 * s],
                        lhsT[0:64, :],
                        lbT[0:64, 4 * g : 4 * g + 4, :],
                        start=True,
                        stop=True,
                    )
                    nc.tensor.matmul(
                        ps[:, 2 * s + 1],
                        lhsT[64:128, :],
                        lbT[64:128, 4 * g : 4 * g + 4, :],
                        start=True,
                        stop=True,
                    )
                # evac dg j: banks 4*(j%2) .. +4 -> out cols
                # [1024j, 1024j+1024) (h0) and [4096+1024j, ...) (h1)
                src = ps[:, 4 * (j % 2) : 4 * (j % 2) + 4, :].rearrange(
                    "p (s h) i -> p h s i", s=2, h=2
                )
                dst = out_sb[:, :, j, :].rearrange("p h (s i) -> p h s i", s=2)
                # t=0: ACT is busy with Ln chunks early; give DVE j=0,1
                if t == 0:
                    use_dve = j < 2
                else:
                    use_dve = (j % 2) == 0
                if out_mode == "i8":
                    if use_dve:
                        nc.vector.tensor_scalar(
                            dst, src, qp[:, 0:1], b_t, ALU.mult, ALU.add
                        )
                    else:
                        nc.scalar.activation(
                            dst, src, AF.Identity, bias=b_t, scale=qp[:, 0:1]
                        )
                else:
                    if use_dve:
                        nc.vector.tensor_scalar(
                            dst, src, 1.0, b_t, ALU.mult, ALU.add
                        )
                    else:
                        nc.scalar.activation(
                            dst, src, AF.Identity, bias=b_t, scale=1.0
                        )
                nc.sync.dma_start(out=out_r[t, :, :, j, :], in_=out_sb[:, :, j, :])
    nc.compile()
    return nc


def _prep(a, b):
    """Host-side layout prep + quantization range (rigorous bounds)."""
    a32 = np.ascontiguousarray(np.asarray(a, dtype=np.float32))
    b32 = np.ascontiguousarray(np.asarray(b, dtype=np.float32))

    la_h = np.log(a32)                      # (N, D)
    lb_h = np.log(b32)                      # (M, D)
    ent_h = np.einsum("nd,nd->n", a32, la_h)

    # rigorous bounds on out = ent[n] - a[n].lb[m]  (a >= 0)
    colmax = lb_h.max(axis=0)               # (D,)
    colmin = lb_h.min(axis=0)
    lo = float((ent_h - a32 @ colmax).min())
    hi_cs = ent_h + np.linalg.norm(a32, axis=1) * float(
        np.linalg.norm(lb_h, axis=1).max()
    )
    hi_cm = ent_h - a32 @ colmin
    hi = float(np.minimum(hi_cs, hi_cm).max())
    pad = 0.2 + 1e-3 * (hi - lo)            # device numeric drift margin
    lo -= pad
    hi += pad
    inv = 253.0 / (hi - lo)
    cst = -lo * inv - 126.5

    # bT[q, k, j] = b[(k + 32*(q>=64))*128 + j, q%64]
    bT = (
        b32.reshape(2, 32, 128, D)
        .transpose(0, 3, 1, 2)
        .reshape(128, K2, 128)
        .astype(np.float16)
    )
    bT = np.ascontiguousarray(bT)

    qp = np.empty((128, 2), np.float32)
    qp[:, 0] = inv
    qp[:, 1] = cst

    in_maps = []
    for i in range(NCORES):
        A = a32[i * NSHARD : (i + 1) * NSHARD]
        # aT[q, t, j] = -A[t*128 + j, q%64], duplicated across halves
        aTh = (-A).reshape(NT, 128, D).transpose(2, 0, 1).astype(np.float16)
        aT = np.ascontiguousarray(
            np.concatenate([aTh, aTh], axis=0)
        )  # [128, NT, 128]
        an = np.ascontiguousarray(
            A.reshape(NT, 128, D).transpose(1, 0, 2)
        )  # [128, NT, 64] fp32
        in_maps.append({"aT": aT, "an": an, "bT": bT, "qp": qp})
    return in_maps, inv, lo


def _run(a, b, trace=False):
    from concourse.bass_utils import run_bass_kernel_spmd

    if OUT_MODE not in _CACHE:
        _CACHE[OUT_MODE] = _build(OUT_MODE)
    nc = _CACHE[OUT_MODE]
    in_maps, inv, lo = _prep(a, b)
    res = run_bass_kernel_spmd(nc, in_maps, list(range(NCORES)), trace=trace)
    outs = []
    for i, r in enumerate(res.results):
        q = np.asarray(r["out"])
        if OUT_MODE == "i8":
            outs.append((q.astype(np.float32) + 126.5) / inv + lo)
        else:
            outs.append(q.astype(np.float32))
    return np.concatenate(outs, axis=0), res


def kernel(a, b):
    out, _ = _run(a, b, trace=False)
    return out


# revision 5
# speedup vs baseline: 1.2197x; 1.2197x over previous
"""Pairwise KL divergence kernel for Trainium2, SPMD across 8 NeuronCores.

out[n, m] = sum_d a[n,d]*(log a[n,d] - log b[m,d])
          = ent[n] - (a @ log(b)^T)[n, m],  ent = rowsum(a * log a)

Sharding: a (and output rows) split 8 ways; b replicated.
Per core: a_shard (1024, 64), b (8192, 64) -> out_shard (1024, 8192).

Design (v2): the kernel is evac-bound, so everything else is stripped:
  - Host passes layout-permuted fp16 operands so ALL loads are contiguous
    big-descriptor DMAs and NO on-device transposes are needed:
      aT: a_shard^T, negated, duplicated on both partition halves
      bT: b^T with m-tiles 0..31 on partitions 0..63 (d axis) and m-tiles
          32..63 on partitions 64..127 -> after Ln this is the GEMM rhs
      an: a_shard in natural p-major layout (for the entropy term)
  - Device: lbT = Ln(bT) [ACT]; ent chain [ACT+DVE]; then 8 n-tiles x
    8 groups of dual half-array matmuls (K=64 stacked on both partition
    halves, concurrent on disjoint row groups); psum = -cross.
  - Output is stored as int8 with an affine code: q = rne(psum*inv + B),
    B[p,t] = ent*inv + cst (computed on device from qp input). Host
    dequantizes. The quant range [lo, hi] is a rigorous bound computed on
    the host (Cauchy-Schwarz + per-column extrema), so saturation never
    triggers; int8 + RNE gives ~0.4% rel err vs the 2e-2 gate.
  - Evac (the bottleneck: PSUM fp32 reads run at 1 elem/cycle/partition)
    is fused mult+add+quantize, FD=2048 per instruction, split DVE/ACT.
  - int8 output halves HBM store traffic vs fp16 (8 MiB/core).
"""

import numpy as np

N, M, D = 8192, 8192, 64
NCORES = 8
NSHARD = N // NCORES          # 1024 rows of a per core
NT = NSHARD // 128            # 8 n-tiles per core
K2 = M // 128 // 2            # 32 m-tile pairs (h0: tiles 0..31, h1: 32..63)
G = 8                         # groups per n-tile; each = 1024 out cols
NCH = 4                       # bT load/Ln chunks (8 pairs each)

OUT_MODE = "i8"               # "i8" | "f16"

_CACHE = {}


def _build(out_mode):
    from contextlib import ExitStack

    import concourse.bacc as bacc_mod
    import concourse.bass as bass
    import concourse.mybir as mybir
    import concourse.tile as tile

    FP32 = mybir.dt.float32
    F16 = mybir.dt.float16
    I8 = mybir.dt.int8
    AF = mybir.ActivationFunctionType
    ALU = mybir.AluOpType
    AX = mybir.AxisListType
    DT_OUT = I8 if out_mode == "i8" else F16

    nc = bacc_mod.Bacc()
    aT_d = nc.dram_tensor("aT", [128, NT, 128], F16, kind="ExternalInput")
    an_d = nc.dram_tensor("an", [128, NT, D], FP32, kind="ExternalInput")
    bT_d = nc.dram_tensor("bT", [128, K2, 128], F16, kind="ExternalInput")
    qp_d = nc.dram_tensor("qp", [128, 2], FP32, kind="ExternalInput")
    out_d = nc.dram_tensor("out", [NSHARD, M], DT_OUT, kind="ExternalOutput")

    with tile.TileContext(nc) as tc, ExitStack() as ctx:
        apool = ctx.enter_context(tc.tile_pool(name="apool", bufs=1))
        bpool = ctx.enter_context(tc.tile_pool(name="bpool", bufs=NCH))
        lbtp = ctx.enter_context(tc.tile_pool(name="lbtp", bufs=1))
        mmps = ctx.enter_context(tc.tile_pool(name="mmps", bufs=1, space="PSUM"))
        stage = ctx.enter_context(tc.tile_pool(name="stage", bufs=2))

        # b chunks: small first chunk so the first Ln/matmul starts ASAP
        CHUNKS = [(0, 4), (4, 18), (18, 32)]
        # -------- input DMAs, all issued up front on the sync sequencer ----
        b_raws = []
        for c0, c1 in CHUNKS:
            b_raw = bpool.tile([128, c1 - c0, 128], F16, tag="b_raw")
            b_raws.append(b_raw)
        nc.sync.dma_start(out=b_raws[0], in_=bT_d[:, CHUNKS[0][0] : CHUNKS[0][1], :])
        aT = apool.tile([128, NT, 128], F16)
        nc.sync.dma_start(out=aT, in_=aT_d[:, :, :])
        an = apool.tile([128, NT, D], FP32)
        nc.sync.dma_start(out=an, in_=an_d[:, :, :])
        qp = apool.tile([128, 2], FP32)
        nc.sync.dma_start(out=qp, in_=qp_d[:, :])
        for c in range(1, len(CHUNKS)):
            c0, c1 = CHUNKS[c]
            nc.sync.dma_start(out=b_raws[c], in_=bT_d[:, c0:c1, :])

        lbT = lbtp.tile([128, K2, 128], F16)
        la = apool.tile([128, NT, D], FP32)
        prod = apool.tile([128, NT, D], FP32)
        ent = apool.tile([128, NT], FP32)
        bias = apool.tile([128, NT], FP32)

        def bias_chain(ts):
            """ent/bias for tile slice ts (DVE ops; la done on ACT)."""
            nc.vector.tensor_mul(prod[:, ts], an[:, ts], la[:, ts])
            for t in range(ts.start, ts.stop):
                nc.vector.reduce_sum(ent[:, t : t + 1], prod[:, t, :], axis=AX.X)
            if out_mode == "i8":
                nc.vector.tensor_scalar(
                    bias[:, ts], ent[:, ts], qp[:, 0:1], qp[:, 1:2],
                    ALU.mult, ALU.add,
                )
            else:
                nc.vector.tensor_copy(bias[:, ts], ent[:, ts])

        # ACT program: Ln chunk0, tile-0 Ln(a), remaining chunks, rest Ln(a)
        nc.scalar.activation(lbT[:, 0:4, :], b_raws[0], AF.Ln)
        nc.scalar.activation(la[:, 0:1], an[:, 0:1], AF.Ln)
        for c in range(1, len(CHUNKS)):
            c0, c1 = CHUNKS[c]
            nc.scalar.activation(lbT[:, c0:c1, :], b_raws[c], AF.Ln)
        nc.scalar.activation(la[:, 1:NT], an[:, 1:NT], AF.Ln)

        # DVE program: tile-0 bias chain first (needed by first evac)
        bias_chain(slice(0, 1))

        # ---------------- main GEMM + fused evac ----------------
        # psum: all 8 banks as one tile; group g -> slot s=g%4 (banks 2s,2s+1)
        ps = mmps.tile([128, 8, 512], FP32)
        # out HBM: (t p) (h g i) with h=2 halves, g=8 groups, i=512
        out_r = out_d[:, :].rearrange(
            "(t p) (h g i) -> t p h g i", p=128, h=2, i=512
        )

        def evac(dst, src, b_t, use_dve):
            if out_mode == "i8":
                if use_dve:
                    nc.vector.tensor_scalar(
                        dst, src, qp[:, 0:1], b_t, ALU.mult, ALU.add
                    )
                else:
                    nc.scalar.activation(
                        dst, src, AF.Identity, bias=b_t, scale=qp[:, 0:1]
                    )
            else:
                if use_dve:
                    nc.vector.tensor_scalar(dst, src, 1.0, b_t, ALU.mult, ALU.add)
                else:
                    nc.scalar.activation(dst, src, AF.Identity, bias=b_t, scale=1.0)

        for t in range(NT):
            out_sb = stage.tile([128, 2, 8, 512], DT_OUT, tag="out_sb")
            lhsT = aT[:, t, :]
            b_t = bias[:, t : t + 1]
            for g in range(G):
                s = g % 4
                # h0: partitions 0-63 (m = [512g, 512g+512)),
                # h1: partitions 64-127 (m = [4096+512g, ...))
                nc.tensor.matmul(
                    ps[:, 2 * s],
                    lhsT[0:64, :],
                    lbT[0:64, 4 * g : 4 * g + 4, :],
                    start=True,
                    stop=True,
                )
                nc.tensor.matmul(
                    ps[:, 2 * s + 1],
                    lhsT[64:128, :],
                    lbT[64:128, 4 * g : 4 * g + 4, :],
                    start=True,
                    stop=True,
                )
                # fused affine+quantize evac, FD=1024 (one group, 2 banks)
                use_dve = (g < 4) if t == 0 else (g % 2 == 0)
                evac(out_sb[:, :, g, :], ps[:, 2 * s : 2 * s + 2, :], b_t, use_dve)
                if g % 2 == 1:
                    j = g // 2
                    nc.sync.dma_start(
                        out=out_r[t, :, :, 2 * j : 2 * j + 2, :],
                        in_=out_sb[:, :, 2 * j : 2 * j + 2, :],
                    )
                if t == 0 and g == 3:
                    # rest of the bias chain in the shadow of tile 0
                    bias_chain(slice(1, NT))
    nc.compile()
    return nc


def _prep(a, b):
    """Host-side layout prep + quantization range (rigorous bounds)."""
    a32 = np.ascontiguousarray(np.asarray(a, dtype=np.float32))
    b32 = np.ascontiguousarray(np.asarray(b, dtype=np.float32))

    la_h = np.log(a32)                      # (N, D)
    lb_h = np.log(b32)                      # (M, D)
    ent_h = np.einsum("nd,nd->n", a32, la_h)

    # rigorous bounds on out = ent[n] - a[n].lb[m]  (a >= 0)
    colmax = lb_h.max(axis=0)               # (D,)
    colmin = lb_h.min(axis=0)
    lo = float((ent_h - a32 @ colmax).min())
    hi_cs = ent_h + np.linalg.norm(a32, axis=1) * float(
        np.linalg.norm(lb_h, axis=1).max()
    )
    hi_cm = ent_h - a32 @ colmin
    hi = float(np.minimum(hi_cs, hi_cm).max())
    pad = 0.2 + 1e-3 * (hi - lo)            # device numeric drift margin
    lo -= pad
    hi += pad
    inv = 253.0 / (hi - lo)
    cst = -lo * inv - 126.5

    # bT[q, k, j] = b[(k + 32*(q>=64))*128 + j, q%64]
    bT = (
        b32.reshape(2, 32, 128, D)
        .transpose(0, 3, 1, 2)
        .reshape(128, K2, 128)
        .astype(np.float16)
    )
    bT = np.ascontiguousarray(bT)

    qp = np.empty((128, 2), np.float32)
    qp[:, 0] = inv
    qp[:, 1] = cst

    in_maps = []
    for i in range(NCORES):
        A = a32[i * NSHARD : (i + 1) * NSHARD]
        # aT[q, t, j] = -A[t*128 + j, q%64], duplicated across halves
        aTh = (-A).reshape(NT, 128, D).transpose(2, 0, 1).astype(np.float16)
        aT = np.ascontiguousarray(
            np.concatenate([aTh, aTh], axis=0)
        )  # [128, NT, 128]
        an = np.ascontiguousarray(
            A.reshape(NT, 128, D).transpose(1, 0, 2)
        )  # [128, NT, 64] fp32
        in_maps.append({"aT": aT, "an": an, "bT": bT, "qp": qp})
    return in_maps, inv, lo


def _run(a, b, trace=False):
    from concourse.bass_utils import run_bass_kernel_spmd

    if OUT_MODE not in _CACHE:
        _CACHE[OUT_MODE] = _build(OUT_MODE)
    nc = _CACHE[OUT_MODE]
    in_maps, inv, lo = _prep(a, b)
    res = run_bass_kernel_spmd(nc, in_maps, list(range(NCORES)), trace=trace)
    outs = []
    for i, r in enumerate(res.results):
        q = np.asarray(r["out"])
        if OUT_MODE == "i8":
            outs.append((q.astype(np.float32) + 126.5) / inv + lo)
        else:
            outs.append(q.astype(np.float32))
    return np.concatenate(outs, axis=0), res


def kernel(a, b):
    out, _ = _run(a, b, trace=False)
    return out


# revision 8
# speedup vs baseline: 1.2515x; 1.0261x over previous
"""Pairwise KL divergence kernel for Trainium2, SPMD across 8 NeuronCores.

out[n, m] = sum_d a[n,d]*(log a[n,d] - log b[m,d])
          = ent[n] - (a @ log(b)^T)[n, m],  ent = rowsum(a * log a)

Sharding: a (and output rows) split 8 ways; b replicated.
Per core: a_shard (1024, 64), b (8192, 64) -> out_shard (1024, 8192).

Design (v2): the kernel is evac-bound, so everything else is stripped:
  - Host passes layout-permuted fp16 operands so ALL loads are contiguous
    big-descriptor DMAs and NO on-device transposes are needed:
      aT: a_shard^T, negated, duplicated on both partition halves
      bT: b^T with m-tiles 0..31 on partitions 0..63 (d axis) and m-tiles
          32..63 on partitions 64..127 -> after Ln this is the GEMM rhs
      an: a_shard in natural p-major layout (for the entropy term)
  - Device: lbT = Ln(bT) [ACT]; ent chain [ACT+DVE]; then 8 n-tiles x
    8 groups of dual half-array matmuls (K=64 stacked on both partition
    halves, concurrent on disjoint row groups); psum = -cross.
  - Output is stored as int8 with an affine code: q = rne(psum*inv + B),
    B[p,t] = ent*inv + cst (computed on device from qp input). Host
    dequantizes. The quant range [lo, hi] is a rigorous bound computed on
    the host (Cauchy-Schwarz + per-column extrema), so saturation never
    triggers; int8 + RNE gives ~0.4% rel err vs the 2e-2 gate.
  - Evac (the bottleneck: PSUM fp32 reads run at 1 elem/cycle/partition)
    is fused mult+add+quantize, FD=2048 per instruction, split DVE/ACT.
  - int8 output halves HBM store traffic vs fp16 (8 MiB/core).
"""

import numpy as np

N, M, D = 8192, 8192, 64
NCORES = 8
NSHARD = N // NCORES          # 1024 rows of a per core
NT = NSHARD // 128            # 8 n-tiles per core
K2 = M // 128 // 2            # 32 m-tile pairs (h0: tiles 0..31, h1: 32..63)
G = 8                         # groups per n-tile; each = 1024 out cols
NCH = 4                       # bT load/Ln chunks (8 pairs each)

OUT_MODE = "i8"               # "i8" | "f16"

_CACHE = {}


def _build(out_mode):
    from contextlib import ExitStack

    import concourse.bacc as bacc_mod
    import concourse.bass as bass
    import concourse.mybir as mybir
    import concourse.tile as tile

    FP32 = mybir.dt.float32
    F16 = mybir.dt.float16
    I8 = mybir.dt.int8
    AF = mybir.ActivationFunctionType
    ALU = mybir.AluOpType
    AX = mybir.AxisListType
    DT_OUT = I8 if out_mode == "i8" else F16

    nc = bacc_mod.Bacc()
    aT_d = nc.dram_tensor("aT", [128, NT, 128], F16, kind="ExternalInput")
    an_d = nc.dram_tensor("an", [128, NT, D], FP32, kind="ExternalInput")
    bT_d = nc.dram_tensor("bT", [128, K2, 128], F16, kind="ExternalInput")
    qp_d = nc.dram_tensor("qp", [128, 2], FP32, kind="ExternalInput")
    out_d = nc.dram_tensor("out", [NSHARD, M], DT_OUT, kind="ExternalOutput")

    with tile.TileContext(nc) as tc, ExitStack() as ctx:
        apool = ctx.enter_context(tc.tile_pool(name="apool", bufs=1))
        bpool = ctx.enter_context(tc.tile_pool(name="bpool", bufs=NCH))
        lbtp = ctx.enter_context(tc.tile_pool(name="lbtp", bufs=1))
        mmps = ctx.enter_context(tc.tile_pool(name="mmps", bufs=1, space="PSUM"))
        stage = ctx.enter_context(tc.tile_pool(name="stage", bufs=4))

        # b chunks: small first chunk so the first Ln/matmul starts ASAP
        CHUNKS = [(0, 4), (4, 18), (18, 32)]
        # -------- input DMAs, all issued up front on the sync sequencer ----
        b_raws = []
        for c0, c1 in CHUNKS:
            b_raw = bpool.tile([128, c1 - c0, 128], F16, tag="b_raw")
            b_raws.append(b_raw)
        nc.sync.dma_start(out=b_raws[0], in_=bT_d[:, CHUNKS[0][0] : CHUNKS[0][1], :])
        an = apool.tile([128, NT, D], FP32)
        nc.sync.dma_start(out=an, in_=an_d[:, :, :])
        aT = apool.tile([128, NT, 128], F16)
        nc.sync.dma_start(out=aT, in_=aT_d[:, :, :])
        qp = apool.tile([128, 2], FP32)
        nc.sync.dma_start(out=qp, in_=qp_d[:, :])
        for c in range(1, len(CHUNKS)):
            c0, c1 = CHUNKS[c]
            nc.sync.dma_start(out=b_raws[c], in_=bT_d[:, c0:c1, :])

        lbT = lbtp.tile([128, K2, 128], F16)
        la = apool.tile([128, NT, D], FP32)
        prod = apool.tile([128, NT, D], FP32)
        ent = apool.tile([128, NT], FP32)
        bias = apool.tile([128, NT], FP32)

        def bias_chain(ts):
            """ent/bias for tile slice ts (DVE ops; la done on ACT)."""
            nc.vector.tensor_mul(prod[:, ts], an[:, ts], la[:, ts])
            for t in range(ts.start, ts.stop):
                nc.vector.reduce_sum(ent[:, t : t + 1], prod[:, t, :], axis=AX.X)
            if out_mode == "i8":
                nc.vector.tensor_scalar(
                    bias[:, ts], ent[:, ts], qp[:, 0:1], qp[:, 1:2],
                    ALU.mult, ALU.add,
                )
            else:
                nc.vector.tensor_copy(bias[:, ts], ent[:, ts])

        # ACT program: Ln chunk0, tile-0 Ln(a), remaining chunks, rest Ln(a)
        nc.scalar.activation(lbT[:, 0:4, :], b_raws[0], AF.Ln)
        nc.scalar.activation(la[:, 0:1], an[:, 0:1], AF.Ln)
        for c in range(1, len(CHUNKS)):
            c0, c1 = CHUNKS[c]
            nc.scalar.activation(lbT[:, c0:c1, :], b_raws[c], AF.Ln)
        nc.scalar.activation(la[:, 1:NT], an[:, 1:NT], AF.Ln)

        # DVE program: tile-0 bias chain first (needed by first evac)
        bias_chain(slice(0, 1))

        # ---------------- main GEMM + fused evac ----------------
        # psum: all 8 banks as one tile; group g -> slot s=g%4 (banks 2s,2s+1)
        ps = mmps.tile([128, 8, 512], FP32)
        # out HBM: (t p) (h g i) with h=2 halves, g=8 groups, i=512
        out_r = out_d[:, :].rearrange(
            "(t p) (h g i) -> t p h g i", p=128, h=2, i=512
        )

        def evac(dst, src, b_t, use_dve):
            if out_mode == "i8":
                if use_dve:
                    nc.vector.tensor_scalar(
                        dst, src, qp[:, 0:1], b_t, ALU.mult, ALU.add
                    )
                else:
                    nc.scalar.activation(
                        dst, src, AF.Identity, bias=b_t, scale=qp[:, 0:1]
                    )
            else:
                if use_dve:
                    nc.vector.tensor_scalar(dst, src, 1.0, b_t, ALU.mult, ALU.add)
                else:
                    nc.scalar.activation(dst, src, AF.Identity, bias=b_t, scale=1.0)

        for t in range(NT):
            out_sb = stage.tile([128, 2, 8, 512], DT_OUT, tag="out_sb")
            lhsT = aT[:, t, :]
            b_t = bias[:, t : t + 1]
            for g in range(G):
                s = g % 4
                # h0: partitions 0-63 (m = [512g, 512g+512)),
                # h1: partitions 64-127 (m = [4096+512g, ...))
                nc.tensor.matmul(
                    ps[:, 2 * s],
                    lhsT[0:64, :],
                    lbT[0:64, 4 * g : 4 * g + 4, :],
                    start=True,
                    stop=True,
                )
                nc.tensor.matmul(
                    ps[:, 2 * s + 1],
                    lhsT[64:128, :],
                    lbT[64:128, 4 * g : 4 * g + 4, :],
                    start=True,
                    stop=True,
                )
                # fused affine+quantize evac, FD=1024 (one group, 2 banks).
                # Static slot->engine map (slots 0,1 DVE; 2,3 ACT) keeps each
                # matmul's psum-free wait on a single fixed semaphore.
                use_dve = s < 2
                evac(out_sb[:, :, g, :], ps[:, 2 * s : 2 * s + 2, :], b_t, use_dve)
                if g % 4 == 3:
                    j = g // 4
                    nc.sync.dma_start(
                        out=out_r[t, :, :, 4 * j : 4 * j + 4, :],
                        in_=out_sb[:, :, 4 * j : 4 * j + 4, :],
                    )
                if t == 0 and g == 3:
                    # rest of the bias chain in the shadow of tile 0
                    bias_chain(slice(1, NT))
    nc.compile()
    return nc


def _prep(a, b):
    """Host-side layout prep + quantization range (rigorous bounds)."""
    a32 = np.ascontiguousarray(np.asarray(a, dtype=np.float32))
    b32 = np.ascontiguousarray(np.asarray(b, dtype=np.float32))

    la_h = np.log(a32)                      # (N, D)
    lb_h = np.log(b32)                      # (M, D)
    ent_h = np.einsum("nd,nd->n", a32, la_h)

    # rigorous bounds on out = ent[n] - a[n].lb[m]  (a >= 0)
    colmax = lb_h.max(axis=0)               # (D,)
    colmin = lb_h.min(axis=0)
    lo = float((ent_h - a32 @ colmax).min())
    hi_cs = ent_h + np.linalg.norm(a32, axis=1) * float(
        np.linalg.norm(lb_h, axis=1).max()
    )
    hi_cm = ent_h - a32 @ colmin
    hi = float(np.minimum(hi_cs, hi_cm).max())
    pad = 0.2 + 1e-3 * (hi - lo)            # device numeric drift margin
    lo -= pad
    hi += pad
    inv = 253.0 / (hi - lo)
    cst = -lo * inv - 126.5

    # bT[q, k, j] = b[(k + 32*(q>=64))*128 + j, q%64]
    bT = (
        b32.reshape(2, 32, 128, D)
        .transpose(0, 3, 1, 2)
        .reshape(128, K2, 128)
        .astype(np.float16)
    )
    bT = np.ascontiguousarray(bT)

    qp = np.empty((128, 2), np.float32)
    qp[:, 0] = inv
    qp[:, 1] = cst

    in_maps = []
    for i in range(NCORES):
        A = a32[i * NSHARD : (i + 1) * NSHARD]
        # aT[q, t, j] = -A[t*128 + j, q%64], duplicated across halves
        aTh = (-A).reshape(NT, 128, D).transpose(2, 0, 1).astype(np.float16)
        aT = np.ascontiguousarray(
            np.concatenate([aTh, aTh], axis=0)
        )  # [128, NT, 128]
        an = np.ascontiguousarray(
            A.reshape(NT, 128, D).transpose(1, 0, 2)
        )  # [128, NT, 64] fp32
        in_maps.append({"aT": aT, "an": an, "bT": bT, "qp": qp})
    return in_maps, inv, lo


def _run(a, b, trace=False):
    from concourse.bass_utils import run_bass_kernel_spmd

    if OUT_MODE not in _CACHE:
        _CACHE[OUT_MODE] = _build(OUT_MODE)
    nc = _CACHE[OUT_MODE]
    in_maps, inv, lo = _prep(a, b)
    res = run_bass_kernel_spmd(nc, in_maps, list(range(NCORES)), trace=trace)
    outs = []
    for i, r in enumerate(res.results):
        q = np.asarray(r["out"])
        if OUT_MODE == "i8":
            outs.append((q.astype(np.float32) + 126.5) / inv + lo)
        else:
            outs.append(q.astype(np.float32))
    return np.concatenate(outs, axis=0), res


def kernel(a, b):
    out, _ = _run(a, b, trace=False)
    return out
